# revision 2
# baseline (speedup 1.0000x reference)
"""Trainium2 Bass kernel for the degenerate capsule-routing module.

Math: in the reference, `cw = softmax(rw, axis=1)` is immediately summed over
axis 1, so `s == 1` in every routing iteration and the routing loop is a
no-op.  The output reduces exactly to

    out[b, j, d] = ((sum_t x[b, t, :]) @ W).reshape(B, 32, 64)

Strategy: shard the 1024-wide feature (contraction) axis across the 8 cores
(128 features each).  The host pre-transposes each core's shard to
[features=128 partitions, B*T tokens] so the on-device token reduction is a
pure free-axis `reduce_sum` on the vector engine, streamed behind the DMAs.
Each core then multiplies its xsumT [128, 32] slice with its W slice
[128, 2048] on the tensor engine (one K-chunk), producing a partial [32, 2048]
output; the host sums the 8 partials (the contraction all-reduce) and
reshapes.
"""

import sys

if "/opt/trn_rl_repo" not in sys.path:
    sys.path.insert(0, "/opt/trn_rl_repo")

import numpy as np

B, T, I, O = 32, 2048, 1024, 2048
NCAP, DC = 32, 64
NUM_CORES = 8
FPC = I // NUM_CORES  # features per core

# batches per x DMA chunk (must sum to B); small tail chunks shorten the
# critical path between the last DMA byte and the final matmul
CHUNKS = [4, 4, 4, 4, 4, 4, 4, 2, 1, 1]

_cache = {}


def _build():
    import concourse.bacc as bacc
    import concourse.bass as bass
    import concourse.mybir as mybir
    import concourse.tile as tile

    dt = mybir.dt.float32
    nc = bacc.Bacc(
        "TRN2", target_bir_lowering=False, debug=False, num_devices=NUM_CORES
    )
    xs = nc.dram_tensor("xs", [FPC, B * T], dt, kind="ExternalInput").ap()
    w = nc.dram_tensor("w", [FPC, O], dt, kind="ExternalInput").ap()
    out = nc.dram_tensor("out", [B, O], dt, kind="ExternalOutput").ap()

    with tile.TileContext(nc) as tc:
        with (
            tc.tile_pool(name="xt", bufs=3) as xpool,
            tc.tile_pool(name="persist", bufs=1) as ppool,
            tc.tile_pool(name="psum", bufs=4, space=bass.MemorySpace.PSUM) as pspool,
        ):
            wt = ppool.tile([FPC, O], dt, tag="w")
            nc.scalar.dma_start(wt[:], w[:])
            xsumT = ppool.tile([FPC, B], dt, tag="xsum")

            base = 0
            for bc in CHUNKS:
                xt = xpool.tile([FPC, bc * T], dt, tag="x")
                nc.sync.dma_start(xt[:], xs[:, base * T : (base + bc) * T])
                for j in range(bc):
                    nc.vector.reduce_sum(
                        xsumT[:, base + j : base + j + 1],
                        xt[:, j * T : (j + 1) * T],
                        axis=mybir.AxisListType.X,
                    )
                base += bc
            assert base == B

            outsb = ppool.tile([B, O], dt, tag="out")
            for j in range(4):
                ps = pspool.tile([B, 512], dt, tag="ps")
                nc.tensor.matmul(
                    ps[:],
                    xsumT[:],
                    wt[:, j * 512 : (j + 1) * 512],
                    start=True,
                    stop=True,
                )
                nc.vector.tensor_copy(outsb[:, j * 512 : (j + 1) * 512], ps[:])
            nc.sync.dma_start(out[:], outsb[:])

    nc.compile()
    return nc


def _get_runner():
    """Build the Bass program + a persistently-jitted 8-core PJRT runner."""
    if "runner" in _cache:
        return _cache["runner"]

    import jax
    from jax.experimental.shard_map import shard_map
    from jax.sharding import Mesh, PartitionSpec

    from concourse import bass2jax, mybir

    nc = _build()
    bass2jax.install_neuronx_cc_hook()

    partition_name = nc.partition_id_tensor.name if nc.partition_id_tensor else None
    in_names, out_names, out_avals, zero_outs = [], [], [], []
    for alloc in nc.m.functions[0].allocations:
        if not isinstance(alloc, mybir.MemoryLocationSet):
            continue
        name = alloc.memorylocations[0].name
        if alloc.kind == "ExternalInput":
            if name != partition_name:
                in_names.append(name)
        elif alloc.kind == "ExternalOutput":
            shape = tuple(alloc.tensor_shape)
            dtype = mybir.dt.np(alloc.dtype)
            out_names.append(name)
            out_avals.append(jax.core.ShapedArray(shape, dtype))
            zero_outs.append(np.zeros(shape, dtype))
    n_params = len(in_names)
    n_outs = len(out_names)
    all_names = list(in_names) + list(out_names)
    if partition_name is not None:
        all_names.append(partition_name)

    def _body(*args):
        operands = list(args)
        if partition_name is not None:
            operands.append(bass2jax.partition_id_tensor())
        outs = bass2jax._bass_exec_p.bind(
            *operands,
            out_avals=tuple(out_avals),
            in_names=tuple(all_names),
            out_names=tuple(out_names),
            lowering_input_output_aliases=(),
            sim_require_finite=True,
            sim_require_nnan=True,
            nc=nc,
        )
        return tuple(outs)

    devices = jax.devices()[:NUM_CORES]
    assert len(devices) == NUM_CORES
    mesh = Mesh(np.asarray(devices), ("core",))
    in_specs = (PartitionSpec("core"),) * (n_params + n_outs)
    out_specs = (PartitionSpec("core"),) * n_outs
    sharded = jax.jit(
        shard_map(
            _body, mesh=mesh, in_specs=in_specs, out_specs=out_specs, check_rep=False
        ),
        keep_unused=True,
    )

    runner = dict(
        nc=nc,
        mesh=mesh,
        sharded=sharded,
        in_names=in_names,
        out_names=out_names,
        out_avals=out_avals,
        zero_outs=zero_outs,
        n_params=n_params,
    )
    _cache["runner"] = runner
    return runner


def _shard_inputs(x, W):
    """Host-side shard + relayout: per core c, xs = x[:, :, cs].T flattened
    to [128 feat, B*T tok]; w = W[cs, :]."""
    in_maps = []
    for c in range(NUM_CORES):
        sl = slice(c * FPC, (c + 1) * FPC)
        xs_c = np.ascontiguousarray(x[:, :, sl].transpose(2, 0, 1).reshape(FPC, B * T))
        w_c = np.ascontiguousarray(W[sl, :])
        in_maps.append({"xs": xs_c, "w": w_c})
    return in_maps


def _concat_args(runner, in_maps):
    concat_in = [
        np.concatenate([in_maps[c][name] for c in range(NUM_CORES)], axis=0)
        for name in runner["in_names"]
    ]
    concat_zeros = [
        np.zeros((NUM_CORES * z.shape[0], *z.shape[1:]), z.dtype)
        for z in runner["zero_outs"]
    ]
    return concat_in, concat_zeros


def kernel(x, W):
    x = np.asarray(x, dtype=np.float32)
    W = np.asarray(W, dtype=np.float32)
    assert x.shape == (B, T, I) and W.shape == (I, O)

    runner = _get_runner()
    in_maps = _shard_inputs(x, W)
    concat_in, concat_zeros = _concat_args(runner, in_maps)
    out_arrs = runner["sharded"](*concat_in, *concat_zeros)

    aval = runner["out_avals"][0]
    partials = np.asarray(out_arrs[0]).reshape(NUM_CORES, *aval.shape)
    full = partials.sum(axis=0, dtype=np.float64).astype(np.float32)
    return full.reshape(B, NCAP, DC)


def bench(x, W, iters=48, warmup=4):
    """Amortized per-iteration device time: queue `iters` executions with
    on-device inputs and time the batch.  Returns (per_iter_ns, results)."""
    import time

    import jax
    from jax.sharding import NamedSharding, PartitionSpec

    x = np.asarray(x, dtype=np.float32)
    W = np.asarray(W, dtype=np.float32)
    runner = _get_runner()
    in_maps = _shard_inputs(x, W)
    concat_in, concat_zeros = _concat_args(runner, in_maps)

    sh = NamedSharding(runner["mesh"], PartitionSpec("core"))
    dev_args = [jax.device_put(a, sh) for a in concat_in + concat_zeros]

    fn = runner["sharded"]
    for _ in range(warmup):
        outs = fn(*dev_args)
    jax.block_until_ready(outs)

    t0 = time.perf_counter()
    last = None
    for _ in range(iters):
        last = fn(*dev_args)
    jax.block_until_ready(last)
    t1 = time.perf_counter()
    return (t1 - t0) / iters * 1e9, last


# revision 12
# speedup vs baseline: 27.3828x; 27.3828x over previous
"""Trainium2 Bass kernel for the degenerate capsule-routing module.

Math: in the reference, `cw = softmax(rw, axis=1)` is immediately summed over
axis 1, so `s == 1` in every routing iteration and the routing loop is a
no-op.  The output reduces exactly to

    out[b, j, d] = ((sum_t x[b, t, :]) @ W).reshape(B, 32, 64)

Strategy: shard the 1024-wide feature (contraction) axis across the 8 cores
(128 features each).  The host pre-transposes each core's shard to
[features=128 partitions, B*T tokens] so the on-device token reduction is a
pure free-axis `reduce_sum` on the vector engine, streamed behind the DMAs.
Each core then multiplies its xsumT [128, 32] slice with its W slice
[128, 2048] on the tensor engine (one K-chunk), producing a partial [32, 2048]
output; the host sums the 8 partials (the contraction all-reduce) and
reshapes.
"""

import sys

if "/opt/trn_rl_repo" not in sys.path:
    sys.path.insert(0, "/opt/trn_rl_repo")

import numpy as np

B, T, I, O = 32, 2048, 1024, 2048
NCAP, DC = 32, 64
NUM_CORES = 8
FPC = I // NUM_CORES  # features per core

# batches per x DMA chunk (must sum to B); small tail chunks shorten the
# critical path between the last DMA byte and the final matmul
CHUNKS = (2,) * 14 + (1,) * 4

_cache = {}


def _build(
    repeat=1,
    chunks=CHUNKS,
    xbufs=6,
    dma_cycle=("sync", "scalar"),
    w_engine="gpsimd",
    repeat_full=False,
    no_reduce=False,
    hw_loop=0,
    staggered=False,
    final_f32r=True,
    split_out=True,
):
    """Build the per-core Bass program.

    `repeat` re-streams the x reduction that many times (work amplification
    for HW timing only); with `repeat_full` the final matmul + store are
    repeated too.  `dma_cycle` assigns x-chunk DMAs round-robin to the named
    engines (sync/scalar = the two HWDGE rings, gpsimd = SWDGE).
    """
    import concourse.bacc as bacc
    import concourse.bass as bass
    import concourse.mybir as mybir
    import concourse.tile as tile

    dt = mybir.dt.float32
    nc = bacc.Bacc(
        "TRN2", target_bir_lowering=False, debug=False, num_devices=NUM_CORES
    )
    xs = nc.dram_tensor("xs", [FPC, B * T], dt, kind="ExternalInput").ap()
    w = nc.dram_tensor("w", [FPC, O], dt, kind="ExternalInput").ap()
    out = nc.dram_tensor("out", [B, O], dt, kind="ExternalOutput").ap()

    with tile.TileContext(nc) as tc:
        with (
            tc.tile_pool(name="xt", bufs=xbufs) as xpool,
            tc.tile_pool(name="persist", bufs=1) as ppool,
            tc.tile_pool(name="psum", bufs=4, space=bass.MemorySpace.PSUM) as pspool,
        ):
            wt = ppool.tile([FPC, O], dt, tag="w")
            getattr(nc, w_engine).dma_start(wt[:], w[:])
            xsumT = ppool.tile([FPC, B], dt, tag="xsum")

            if no_reduce:
                nc.gpsimd.memset(xsumT[:], 0.0)

            def stream_pass():
                base = 0
                for ci, bc in enumerate(chunks):
                    eng = getattr(nc, dma_cycle[ci % len(dma_cycle)])
                    xt = xpool.tile([FPC, bc * T], dt, tag="x")
                    eng.dma_start(xt[:], xs[:, base * T : (base + bc) * T])
                    if not no_reduce:
                        for j in range(bc):
                            nc.vector.reduce_sum(
                                xsumT[:, base + j : base + j + 1],
                                xt[:, j * T : (j + 1) * T],
                                axis=mybir.AxisListType.X,
                            )
                    base += bc
                assert base == B

            if final_f32r:
                wt_r = ppool.tile([FPC, O], mybir.dt.float32r, tag="wr")
                nc.vector.tensor_copy(wt_r[:], wt[:])

            def tail_pass():
                outsb = ppool.tile([B, O], dt, tag="out")
                if final_f32r:
                    xsum_r = ppool.tile([FPC, B], mybir.dt.float32r, tag="xsr")
                    nc.vector.tensor_copy(xsum_r[:], xsumT[:])
                    lhs, rhs = xsum_r, wt_r
                else:
                    lhs, rhs = xsumT, wt
                for j in range(4):
                    ps = pspool.tile([B, 512], dt, tag="ps")
                    nc.tensor.matmul(
                        ps[:],
                        lhs[:],
                        rhs[:, j * 512 : (j + 1) * 512],
                        start=True,
                        stop=True,
                    )
                    nc.vector.tensor_copy(outsb[:, j * 512 : (j + 1) * 512], ps[:])
                    if split_out:
                        eng = getattr(nc, dma_cycle[j % len(dma_cycle)])
                        eng.dma_start(
                            out[:, j * 512 : (j + 1) * 512],
                            outsb[:, j * 512 : (j + 1) * 512],
                        )
                if not split_out:
                    nc.sync.dma_start(out[:], outsb[:])

            if hw_loop:
                with tc.For_i(0, hw_loop, 1, staggered_reset=staggered):
                    stream_pass()
                tail_pass()
            elif repeat_full:
                for _ in range(repeat):
                    stream_pass()
                    tail_pass()
            else:
                for _ in range(repeat):
                    stream_pass()
                tail_pass()

    nc.compile()
    return nc


def make_runner(nc):
    """Persistently-jitted 8-core PJRT runner for a compiled Bass program."""
    import jax
    from jax.experimental.shard_map import shard_map
    from jax.sharding import Mesh, PartitionSpec

    from concourse import bass2jax, mybir

    bass2jax.install_neuronx_cc_hook()

    partition_name = nc.partition_id_tensor.name if nc.partition_id_tensor else None
    in_names, out_names, out_avals, zero_outs = [], [], [], []
    for alloc in nc.m.functions[0].allocations:
        if not isinstance(alloc, mybir.MemoryLocationSet):
            continue
        name = alloc.memorylocations[0].name
        if alloc.kind == "ExternalInput":
            if name != partition_name:
                in_names.append(name)
        elif alloc.kind == "ExternalOutput":
            shape = tuple(alloc.tensor_shape)
            dtype = mybir.dt.np(alloc.dtype)
            out_names.append(name)
            out_avals.append(jax.core.ShapedArray(shape, dtype))
            zero_outs.append(np.zeros(shape, dtype))
    n_params = len(in_names)
    n_outs = len(out_names)
    all_names = list(in_names) + list(out_names)
    if partition_name is not None:
        all_names.append(partition_name)

    def _body(*args):
        operands = list(args)
        if partition_name is not None:
            operands.append(bass2jax.partition_id_tensor())
        outs = bass2jax._bass_exec_p.bind(
            *operands,
            out_avals=tuple(out_avals),
            in_names=tuple(all_names),
            out_names=tuple(out_names),
            lowering_input_output_aliases=(),
            sim_require_finite=True,
            sim_require_nnan=True,
            nc=nc,
        )
        return tuple(outs)

    devices = jax.devices()[:NUM_CORES]
    assert len(devices) == NUM_CORES
    mesh = Mesh(np.asarray(devices), ("core",))
    in_specs = (PartitionSpec("core"),) * (n_params + n_outs)
    out_specs = (PartitionSpec("core"),) * n_outs
    sharded = jax.jit(
        shard_map(
            _body, mesh=mesh, in_specs=in_specs, out_specs=out_specs, check_rep=False
        ),
        keep_unused=True,
    )

    return dict(
        nc=nc,
        mesh=mesh,
        sharded=sharded,
        in_names=in_names,
        out_names=out_names,
        out_avals=out_avals,
        zero_outs=zero_outs,
        n_params=n_params,
    )


def _shard_inputs(x, W):
    """Host-side shard + relayout: per core c, xs = x[:, :, cs].T flattened
    to [128 feat, B*T tok]; w = W[cs, :]."""
    in_maps = []
    for c in range(NUM_CORES):
        sl = slice(c * FPC, (c + 1) * FPC)
        xs_c = np.ascontiguousarray(x[:, :, sl].transpose(2, 0, 1).reshape(FPC, B * T))
        w_c = np.ascontiguousarray(W[sl, :])
        in_maps.append({"xs": xs_c, "w": w_c})
    return in_maps


def _concat_args(runner, in_maps):
    concat_in = [
        np.concatenate([in_maps[c][name] for c in range(NUM_CORES)], axis=0)
        for name in runner["in_names"]
    ]
    concat_zeros = [
        np.zeros((NUM_CORES * z.shape[0], *z.shape[1:]), z.dtype)
        for z in runner["zero_outs"]
    ]
    return concat_in, concat_zeros


def kernel(x, W):
    x = np.asarray(x, dtype=np.float32)
    W = np.asarray(W, dtype=np.float32)
    assert x.shape == (B, T, I) and W.shape == (I, O)

    if "runner" not in _cache:
        _cache["runner"] = make_runner(_build())
    runner = _cache["runner"]

    in_maps = _shard_inputs(x, W)
    concat_in, concat_zeros = _concat_args(runner, in_maps)
    out_arrs = runner["sharded"](*concat_in, *concat_zeros)

    aval = runner["out_avals"][0]
    partials = np.asarray(out_arrs[0]).reshape(NUM_CORES, *aval.shape)
    full = partials.sum(axis=0, dtype=np.float64).astype(np.float32)
    return full.reshape(B, NCAP, DC)


def bench_runner(runner, x, W, iters=48, warmup=4):
    """Amortized wall-clock per dispatched execution with on-device inputs."""
    import time

    import jax
    from jax.sharding import NamedSharding, PartitionSpec

    in_maps = _shard_inputs(np.asarray(x, np.float32), np.asarray(W, np.float32))
    concat_in, concat_zeros = _concat_args(runner, in_maps)
    sh = NamedSharding(runner["mesh"], PartitionSpec("core"))
    dev_args = [jax.device_put(a, sh) for a in concat_in + concat_zeros]

    fn = runner["sharded"]
    for _ in range(warmup):
        outs = fn(*dev_args)
    jax.block_until_ready(outs)

    t0 = time.perf_counter()
    last = None
    for _ in range(iters):
        last = fn(*dev_args)
    jax.block_until_ready(last)
    t1 = time.perf_counter()
    return (t1 - t0) / iters * 1e9, last


def bench(x, W, iters=48, warmup=4):
    if "runner" not in _cache:
        _cache["runner"] = make_runner(_build())
    return bench_runner(_cache["runner"], x, W, iters=iters, warmup=warmup)


# revision 17
# speedup vs baseline: 28.1759x; 1.0290x over previous
"""Trainium2 Bass kernel for the degenerate capsule-routing module.

Math: in the reference, `cw = softmax(rw, axis=1)` is immediately summed over
axis 1, so `s == 1` in every routing iteration and the routing loop is a
no-op.  The output reduces exactly to

    out[b, j, d] = ((sum_t x[b, t, :]) @ W).reshape(B, 32, 64)

Strategy: shard the 1024-wide feature (contraction) axis across the 8 cores
(128 features each).  The host pre-transposes each core's shard to
[features=128 partitions, B*T tokens] so the on-device token reduction is a
pure free-axis `reduce_sum` on the vector engine, streamed behind the DMAs.
Each core then multiplies its xsumT [128, 32] slice with its W slice
[128, 2048] on the tensor engine (one K-chunk), producing a partial [32, 2048]
output; the host sums the 8 partials (the contraction all-reduce) and
reshapes.
"""

import sys

if "/opt/trn_rl_repo" not in sys.path:
    sys.path.insert(0, "/opt/trn_rl_repo")

import numpy as np

B, T, I, O = 32, 2048, 1024, 2048
NCAP, DC = 32, 64
NUM_CORES = 8
FPC = I // NUM_CORES  # features per core

# batches per x DMA chunk (must sum to B); small tail chunks shorten the
# critical path between the last DMA byte and the final matmul
CHUNKS = (2,) * 14 + (1,) * 4

_cache = {}


def _build(
    repeat=1,
    chunks=CHUNKS,
    xbufs=6,
    dma_cycle=("sync", "scalar"),
    w_engine="gpsimd",
    repeat_full=False,
    no_reduce=False,
    hw_loop=0,
    staggered=False,
    final_f32r=True,
    split_out=True,
    fused_reduce=True,
    xsum_f32r=True,
    act_cycle=0,
):
    """Build the per-core Bass program.

    `repeat` re-streams the x reduction that many times (work amplification
    for HW timing only); with `repeat_full` the final matmul + store are
    repeated too.  `dma_cycle` assigns x-chunk DMAs round-robin to the named
    engines (sync/scalar = the two HWDGE rings, gpsimd = SWDGE).
    """
    import concourse.bacc as bacc
    import concourse.bass as bass
    import concourse.mybir as mybir
    import concourse.tile as tile

    dt = mybir.dt.float32
    nc = bacc.Bacc(
        "TRN2", target_bir_lowering=False, debug=False, num_devices=NUM_CORES
    )
    xs = nc.dram_tensor("xs", [FPC, B * T], dt, kind="ExternalInput").ap()
    w = nc.dram_tensor("w", [FPC, O], dt, kind="ExternalInput").ap()
    out = nc.dram_tensor("out", [B, O], dt, kind="ExternalOutput").ap()

    with tile.TileContext(nc) as tc:
        with (
            tc.tile_pool(name="xt", bufs=xbufs) as xpool,
            tc.tile_pool(name="persist", bufs=1) as ppool,
            tc.tile_pool(name="psum", bufs=4, space=bass.MemorySpace.PSUM) as pspool,
        ):
            wt = ppool.tile([FPC, O], dt, tag="w")
            getattr(nc, w_engine).dma_start(wt[:], w[:])
            xsum_dt = mybir.dt.float32r if (final_f32r and xsum_f32r) else dt
            xsumT = ppool.tile([FPC, B], xsum_dt, tag="xsum")
            act_scratch = (
                ppool.tile([FPC, T], dt, tag="actscratch") if act_cycle else None
            )

            if no_reduce:
                nc.gpsimd.memset(xsumT[:], 0.0)

            def reduce_chunk(xt, base, bc, on_act):
                if on_act:
                    for j in range(bc):
                        nc.scalar.activation(
                            act_scratch[:],
                            xt[:, j, :],
                            mybir.ActivationFunctionType.Copy,
                            accum_out=xsumT[:, base + j : base + j + 1],
                        )
                elif fused_reduce:
                    # f32r out only rounds the final write; internal
                    # accumulation stays fp32, so low precision is fine here
                    with nc.allow_low_precision(reason="f32r round on final write"):
                        nc.vector.reduce_sum(
                            xsumT[:, base : base + bc].unsqueeze(-1),
                            xt[:],
                            axis=mybir.AxisListType.X,
                        )
                else:
                    for j in range(bc):
                        nc.vector.reduce_sum(
                            xsumT[:, base + j : base + j + 1],
                            xt[:, j, :],
                            axis=mybir.AxisListType.X,
                        )

            def stream_pass():
                base = 0
                for ci, bc in enumerate(chunks):
                    eng = getattr(nc, dma_cycle[ci % len(dma_cycle)])
                    xt = xpool.tile([FPC, bc, T], dt, tag="x")
                    eng.dma_start(xt[:], xs[:, base * T : (base + bc) * T])
                    if not no_reduce:
                        on_act = act_cycle and (ci % act_cycle == act_cycle - 1)
                        reduce_chunk(xt, base, bc, on_act)
                    base += bc
                assert base == B

            if final_f32r:
                wt_r = ppool.tile([FPC, O], mybir.dt.float32r, tag="wr")
                nc.vector.tensor_copy(wt_r[:], wt[:])

            def tail_pass():
                outsb = ppool.tile([B, O], dt, tag="out")
                if final_f32r and not xsum_f32r:
                    xsum_r = ppool.tile([FPC, B], mybir.dt.float32r, tag="xsr")
                    nc.vector.tensor_copy(xsum_r[:], xsumT[:])
                    lhs, rhs = xsum_r, wt_r
                elif final_f32r:
                    lhs, rhs = xsumT, wt_r
                else:
                    lhs, rhs = xsumT, wt
                for j in range(4):
                    ps = pspool.tile([B, 512], dt, tag="ps")
                    nc.tensor.matmul(
                        ps[:],
                        lhs[:],
                        rhs[:, j * 512 : (j + 1) * 512],
                        start=True,
                        stop=True,
                    )
                    nc.vector.tensor_copy(outsb[:, j * 512 : (j + 1) * 512], ps[:])
                    if split_out:
                        eng = getattr(nc, dma_cycle[j % len(dma_cycle)])
                        eng.dma_start(
                            out[:, j * 512 : (j + 1) * 512],
                            outsb[:, j * 512 : (j + 1) * 512],
                        )
                if not split_out:
                    nc.sync.dma_start(out[:], outsb[:])

            if hw_loop:
                with tc.For_i(0, hw_loop, 1, staggered_reset=staggered):
                    stream_pass()
                tail_pass()
            elif repeat_full:
                for _ in range(repeat):
                    stream_pass()
                    tail_pass()
            else:
                for _ in range(repeat):
                    stream_pass()
                tail_pass()

    nc.compile()
    return nc


def make_runner(nc):
    """Persistently-jitted 8-core PJRT runner for a compiled Bass program."""
    import jax
    from jax.experimental.shard_map import shard_map
    from jax.sharding import Mesh, PartitionSpec

    from concourse import bass2jax, mybir

    bass2jax.install_neuronx_cc_hook()

    partition_name = nc.partition_id_tensor.name if nc.partition_id_tensor else None
    in_names, out_names, out_avals, zero_outs = [], [], [], []
    for alloc in nc.m.functions[0].allocations:
        if not isinstance(alloc, mybir.MemoryLocationSet):
            continue
        name = alloc.memorylocations[0].name
        if alloc.kind == "ExternalInput":
            if name != partition_name:
                in_names.append(name)
        elif alloc.kind == "ExternalOutput":
            shape = tuple(alloc.tensor_shape)
            dtype = mybir.dt.np(alloc.dtype)
            out_names.append(name)
            out_avals.append(jax.core.ShapedArray(shape, dtype))
            zero_outs.append(np.zeros(shape, dtype))
    n_params = len(in_names)
    n_outs = len(out_names)
    all_names = list(in_names) + list(out_names)
    if partition_name is not None:
        all_names.append(partition_name)

    def _body(*args):
        operands = list(args)
        if partition_name is not None:
            operands.append(bass2jax.partition_id_tensor())
        outs = bass2jax._bass_exec_p.bind(
            *operands,
            out_avals=tuple(out_avals),
            in_names=tuple(all_names),
            out_names=tuple(out_names),
            lowering_input_output_aliases=(),
            sim_require_finite=True,
            sim_require_nnan=True,
            nc=nc,
        )
        return tuple(outs)

    devices = jax.devices()[:NUM_CORES]
    assert len(devices) == NUM_CORES
    mesh = Mesh(np.asarray(devices), ("core",))
    in_specs = (PartitionSpec("core"),) * (n_params + n_outs)
    out_specs = (PartitionSpec("core"),) * n_outs
    sharded = jax.jit(
        shard_map(
            _body, mesh=mesh, in_specs=in_specs, out_specs=out_specs, check_rep=False
        ),
        keep_unused=True,
    )

    return dict(
        nc=nc,
        mesh=mesh,
        sharded=sharded,
        in_names=in_names,
        out_names=out_names,
        out_avals=out_avals,
        zero_outs=zero_outs,
        n_params=n_params,
    )


def _shard_inputs(x, W):
    """Host-side shard + relayout: per core c, xs = x[:, :, cs].T flattened
    to [128 feat, B*T tok]; w = W[cs, :]."""
    in_maps = []
    for c in range(NUM_CORES):
        sl = slice(c * FPC, (c + 1) * FPC)
        xs_c = np.ascontiguousarray(x[:, :, sl].transpose(2, 0, 1).reshape(FPC, B * T))
        w_c = np.ascontiguousarray(W[sl, :])
        in_maps.append({"xs": xs_c, "w": w_c})
    return in_maps


def _concat_args(runner, in_maps):
    concat_in = [
        np.concatenate([in_maps[c][name] for c in range(NUM_CORES)], axis=0)
        for name in runner["in_names"]
    ]
    concat_zeros = [
        np.zeros((NUM_CORES * z.shape[0], *z.shape[1:]), z.dtype)
        for z in runner["zero_outs"]
    ]
    return concat_in, concat_zeros


def kernel(x, W):
    x = np.asarray(x, dtype=np.float32)
    W = np.asarray(W, dtype=np.float32)
    assert x.shape == (B, T, I) and W.shape == (I, O)

    if "runner" not in _cache:
        _cache["runner"] = make_runner(_build())
    runner = _cache["runner"]

    in_maps = _shard_inputs(x, W)
    concat_in, concat_zeros = _concat_args(runner, in_maps)
    out_arrs = runner["sharded"](*concat_in, *concat_zeros)

    aval = runner["out_avals"][0]
    partials = np.asarray(out_arrs[0]).reshape(NUM_CORES, *aval.shape)
    full = partials.sum(axis=0, dtype=np.float64).astype(np.float32)
    return full.reshape(B, NCAP, DC)


def bench_runner(runner, x, W, iters=48, warmup=4):
    """Amortized wall-clock per dispatched execution with on-device inputs."""
    import time

    import jax
    from jax.sharding import NamedSharding, PartitionSpec

    in_maps = _shard_inputs(np.asarray(x, np.float32), np.asarray(W, np.float32))
    concat_in, concat_zeros = _concat_args(runner, in_maps)
    sh = NamedSharding(runner["mesh"], PartitionSpec("core"))
    dev_args = [jax.device_put(a, sh) for a in concat_in + concat_zeros]

    fn = runner["sharded"]
    for _ in range(warmup):
        outs = fn(*dev_args)
    jax.block_until_ready(outs)

    t0 = time.perf_counter()
    last = None
    for _ in range(iters):
        last = fn(*dev_args)
    jax.block_until_ready(last)
    t1 = time.perf_counter()
    return (t1 - t0) / iters * 1e9, last


def bench(x, W, iters=48, warmup=4):
    if "runner" not in _cache:
        _cache["runner"] = make_runner(_build())
    return bench_runner(_cache["runner"], x, W, iters=iters, warmup=warmup)
